# revision 24
# baseline (speedup 1.0000x reference)
"""Trainium2 Bass kernel for nn_AttnAware (pixnorm->conv1x1 q/k attention + ResnetBlock).

Sharding: 8 cores = 4 batches x 2 query-halves. Each core receives its batch's
x [256, 4096] with pixel columns rotated so that its 2048 query pixels are the
first 2048 columns (attention is permutation-invariant over keys, and all
other ops are per-pixel). Single SPMD program, no collectives.

Per-core data layout: channels on partitions, pixels on free axis.
All PE matmuls run in 16/8-bit (1 cycle/row streaming and cheap LDWEIGHTS;
conv weights are converted to fp16 host-side and shipped in one packed DMA).
Attention works in the S^T orientation: S^T[j,i] tiles [128 keys, i-chunk]
computed as k_block^T @ q in fp16, exp on ACT (scale fused) writing fp8e4 P
pair-tiles, O^T accumulated with fp8 DoubleRow matmuls (two key blocks per
instruction, 0.5 cycles/row) against V^T pre-transposed per head on the PE.
The softmax denominator is accumulated on the PE as fp8 DoubleRow
ones-matmuls over the same P tiles, 1/D comes from the fast approximate DVE
reciprocal, is broadcast across partitions on the Pool engine, and O is
normalized inline per (head, i-pass). Pixelnorm row->tile broadcasts also go
through the Pool engine (partition_broadcast) instead of PE matmuls.
"""

import math
from contextlib import ExitStack

import numpy as np

import concourse.bass as bass
import concourse.mybir as mybir
import concourse.tile as tile
from concourse import bacc
from concourse.masks import make_identity

# ---------------- problem constants (hardcoded per contract) ----------------
B = 4
C = 256
HW = 64
N = HW * HW              # 4096 pixels
NQ = N // 2              # 2048 query pixels per core
NH = 2
HD = C // NH             # 128
CT = C // 128            # 2 channel tiles
C2T = 2 * C // 128       # 4 channel tiles for cat
JB = N // 128            # 32 key blocks
ATT_SCALE = HD ** -0.5
RATIO = 1.0 / (1.0 + 1e-8)   # PartialConv mask ratio (== 1.0f in fp32)
EPS = 1e-8
ISQ2 = 1.0 / math.sqrt(2.0)

# packed weight tile order (14 x [128, 256] fp16 column-tiles)
W_ORDER = [("wqT", CT), ("wkT", CT), ("wsT", C2T), ("w1T", C2T), ("w2T", CT)]
NWT = sum(n for _, n in W_ORDER)
# packed bias/alpha column order (18 x [128, 1] f32)
B_ORDER = [("bq", CT), ("bk", CT), ("b1", CT), ("bsc", CT), ("aq", CT),
           ("ak", CT), ("ar1", C2T), ("ar2", CT)]
NBC = sum(n for _, n in B_ORDER)

# ---------------- tuning knobs ----------------
IW = 1024                # i-columns per attention pass (PSUM S tile width)
LDW_OPT = False          # walrus ldw-opt is incompatible with 16/8-bit ldweights

f32 = mybir.dt.float32
f32r = mybir.dt.float32r
f16 = mybir.dt.float16
f8 = mybir.dt.float8e4
AF = mybir.ActivationFunctionType
OP = mybir.AluOpType
DR = mybir.MatmulPerfMode.DoubleRow


def build_program():
    nc = bacc.Bacc("TRN2", target_bir_lowering=False, debug=False)

    # register the pixnorm epsilon as a const AP usable as an ACT bias
    _eps_t = nc.alloc_sbuf_tensor(f"const-float32-{EPS}", [128, 1], f32)
    nc.gpsimd.memset(_eps_t.ap(), EPS)
    nc.const_aps.aps[(f32, EPS)] = _eps_t.ap()
    nc.all_engine_barrier()

    d = {}
    d["x"] = nc.dram_tensor("x", (C, N), f32, kind="ExternalInput").ap()
    d["wpack"] = nc.dram_tensor("wpack", (128, NWT * 256), f16,
                                kind="ExternalInput").ap()
    d["bpack"] = nc.dram_tensor("bpack", (128, NBC), f32,
                                kind="ExternalInput").ap()
    d["y"] = nc.dram_tensor("y", (C, NQ), f32, kind="ExternalOutput").ap()

    with tile.TileContext(nc) as tc:
        _body(tc, nc, d)
    nc.compile()
    return nc


def _body(tc, nc, d):
    x_d, y_d = d["x"], d["y"]

    with ExitStack() as top:
        const = top.enter_context(tc.tile_pool(name="const", bufs=1))
        wts = top.enter_context(tc.tile_pool(name="wts", bufs=1))

        ident16 = const.tile([128, 128], f16, tag="ident16", name="ident16")
        make_identity(nc, ident16[:])
        ones_col16 = const.tile([128, 1], f16, tag="ones_col16", name="ones_col16")
        nc.vector.memset(ones_col16[:], 1.0)
        ones_dr8 = const.tile([128, 2, 32], f8, tag="ones_dr8", name="ones_dr8")
        nc.vector.memset(ones_dr8[:], 1.0)

        # tensors that live into phase C
        with tc.tile_pool(name="oout", bufs=1) as oout:
            osb = [oout.tile([128, NQ], f16, tag=f"o{h}", name=f"o{h}")
                   for h in range(NH)]
            x16 = [oout.tile([128, N], f16, tag=f"x16_{h}", name=f"x16_{h}")
                   for h in range(NH)]

            # kqv: tensors that live from phase A through attention; closed
            # explicitly before the ResnetBlock pools open to reuse SBUF
            kqv_stack = ExitStack()
            kqv = kqv_stack.enter_context(tc.tile_pool(name="kqv", bufs=1))
            vt = [kqv.tile([128, N], f8, tag=f"vt{h}", name=f"vt{h}") for h in range(NH)]
            kt = [kqv.tile([128, N], f16, tag=f"k{h}", name=f"k{h}") for h in range(NH)]
            qt = [kqv.tile([128, NQ], f16, tag=f"q{h}", name=f"q{h}") for h in range(NH)]

            # x DMA first (critical path): 4 transfers, first pixel-half first
            front_stack = ExitStack()
            front = front_stack.enter_context(tc.tile_pool(name="front", bufs=1))
            xt = [front.tile([128, N], f32, tag=f"x{ct}", name=f"x{ct}")
                  for ct in range(CT)]
            # pixel half 0 on the SP ring, half 1 on the ACT ring (parallel)
            for ct in range(CT):
                nc.sync.dma_start(
                    xt[ct][:, :NQ], x_d[ct * 128:(ct + 1) * 128, :NQ])
            for ct in range(CT):
                nc.scalar.dma_start(
                    xt[ct][:, NQ:], x_d[ct * 128:(ct + 1) * 128, NQ:])

            # packed weights (one DMA) + packed biases/alphas (one DMA)
            wtile = wts.tile([128, NWT * 256], f16, tag="wpack", name="wpack")
            nc.sync.dma_start(wtile[:], d["wpack"])
            btile = wts.tile([128, NBC], f32, tag="bpack", name="bpack")
            nc.sync.dma_start(btile[:], d["bpack"])

            wv = {}
            off = 0
            for nm, n in W_ORDER:
                wv[nm] = [wtile[:, (off + i) * 256:(off + i + 1) * 256]
                          for i in range(n)]
                off += n
            bv = {}
            off = 0
            for nm, n in B_ORDER:
                bv[nm] = [btile[:, off + i:off + i + 1] for i in range(n)]
                off += n
            wqT, wkT, wsT, w1T, w2T = (wv[nm] for nm, _ in W_ORDER)
            bq, bk, b1, bsc, aq, ak, ar1, ar2 = (bv[nm] for nm, _ in B_ORDER)

            # =========== Phase A ===========
            with (
                tc.tile_pool(name="gtmp", bufs=4) as gtmp,
                tc.tile_pool(name="frow", bufs=2) as frow,
                tc.tile_pool(name="bcp", bufs=1) as bcp,
                tc.tile_pool(name="psA", bufs=2, space="PSUM") as psA,
                tc.tile_pool(name="psArow", bufs=1, space="PSUM") as psArow,
            ):
                # fp16 copy of x on the ACT engine (Identity, in every
                # act table set), 1024-col chunks so stats start early
                for q in range(N // 1024):
                    for ct in range(CT):
                        nc.scalar.activation(
                            x16[ct][:, q * 1024:(q + 1) * 1024],
                            xt[ct][:, q * 1024:(q + 1) * 1024],
                            AF.Identity, scale=1.0)

                # pixelnorm stats per half: squares (DVE, fp16), ones-matmul
                # column sums into one [1, 2048] PSUM row, one ln + one exp,
                # then one Pool broadcast to a [128, 2048] fp16 scale tile
                bch = []
                for half in range(2):
                    srow = psArow.tile([1, NQ], f32, tag="srow", name="srow")
                    for cc in range(4):
                        ccg = half * 4 + cc
                        for ct in range(CT):
                            s = gtmp.tile([128, 512], f16, tag="sq", name="sq",
                                          bufs=4)
                            nc.vector.tensor_tensor(
                                s[:], x16[ct][:, ccg * 512:(ccg + 1) * 512],
                                x16[ct][:, ccg * 512:(ccg + 1) * 512],
                                op=OP.mult)
                            nc.tensor.matmul(srow[:, cc * 512:(cc + 1) * 512],
                                             ones_col16[:], s[:],
                                             start=(ct == 0), stop=(ct == CT - 1))
                    lt = frow.tile([1, NQ], f32, tag="lnt", name="lnt")
                    nc.scalar.activation(lt[:], srow[:], AF.Ln, bias=EPS,
                                         scale=1.0 / C)
                    iv = frow.tile([1, NQ], f16, tag="inv", name="inv")
                    nc.scalar.activation(iv[:], lt[:], AF.Exp, scale=-0.5)
                    bc = bcp.tile([128, NQ], f16, tag=f"bc{half}",
                                  name=f"bc{half}")
                    nc.gpsimd.partition_broadcast(bc[:], iv[:])
                    bch.append(bc)

                # per-1024-chunk: xb = x*inv, gelu per conv, conv matmuls;
                # conv output moves alternate DVE / Pool to balance load
                def conv_block(c2, convs):
                    half, bc = c2 // 2, bch[c2 // 2]
                    csl = slice((c2 % 2) * 1024, (c2 % 2 + 1) * 1024)
                    gsl = slice(c2 * 1024, (c2 + 1) * 1024)
                    xb = []
                    for ct in range(CT):
                        t = gtmp.tile([128, 1024], f16, tag="g", name="xb")
                        nc.vector.tensor_tensor(
                            t[:], x16[ct][:, gsl], bc[:, csl], op=OP.mult)
                        xb.append(t)
                    for (wT, alpha, bias, out_tiles, oc0) in convs:
                        gchunks = []
                        for ct in range(CT):
                            g = gtmp.tile([128, 1024], f16, tag="g", name="g")
                            nc.scalar.activation(g[:], xb[ct][:],
                                                 AF.Gelu, scale=alpha[ct])
                            gchunks.append(g)
                        for mo in range(CT):
                            for rr in range(2):
                                ps = psA.tile([128, 512], f32, tag="conv",
                                              name="conv")
                                for kc in range(CT):
                                    nc.tensor.matmul(
                                        ps[:],
                                        wT[kc][:, mo * 128:(mo + 1) * 128],
                                        gchunks[kc][:, rr * 512:(rr + 1) * 512],
                                        start=(kc == 0), stop=(kc == CT - 1))
                                osl = slice(oc0 + (c2 % 2) * 1024 + rr * 512,
                                            oc0 + (c2 % 2) * 1024 + rr * 512 + 512)
                                if mo == 0:
                                    nc.vector.tensor_scalar(
                                        out_tiles[mo][:, osl], ps[:],
                                        bias[mo], None, op0=OP.add)
                                else:
                                    nc.scalar.activation(
                                        out_tiles[mo][:, osl], ps[:],
                                        AF.Identity, bias=bias[mo], scale=1.0)

                for c2 in range(2):
                    conv_block(c2, [(wqT, aq, bq, qt, 0), (wkT, ak, bk, kt, 0)])
                for c2 in range(2, 4):
                    conv_block(c2, [(wkT, ak, bk, kt, NQ)])

                # V^T per head: PE transpose of fp16 x, 4 blocks per PSUM slot,
                # copied out with a cast to fp8 (layout [(jb) d] flat == the
                # [jb/2, 2, d] DoubleRow view). Emitted after the convs so the
                # conv matmuls aren't stuck behind transposes that wait on the
                # second half of the x DMA.
                for h in range(NH):
                    for qb in range(JB // 4):
                        tp = psA.tile([128, 512], f16, tag="tp", name="tp")
                        for rr in range(4):
                            jb = qb * 4 + rr
                            nc.tensor.transpose(
                                tp[:, rr * 128:(rr + 1) * 128],
                                x16[h][:, jb * 128:(jb + 1) * 128], ident16[:])
                        nc.vector.tensor_copy(vt[h][:, qb * 512:(qb + 1) * 512], tp[:])

            front_stack.close()  # xt no longer needed (phase C uses x16)

            # =========== Phase B: attention (fp8 DoubleRow O and D) ==========
            with (
                tc.tile_pool(name="psS", bufs=2, space="PSUM") as psS,
                tc.tile_pool(name="psO", bufs=1, space="PSUM") as psO,
                tc.tile_pool(name="psD", bufs=1, space="PSUM") as psD,
                tc.tile_pool(name="pexp", bufs=3) as pexp,
                tc.tile_pool(name="drow", bufs=2) as drow_pool,
            ):
                NR = IW // 512
                NJP = JB // 2
                for h in range(NH):
                    for ip in range(NQ // IW):
                        i0 = ip * IW
                        o_ps = psO.tile([128, IW], f32, tag="o", name="o")
                        d_ps = psD.tile([32, IW], f32, tag="d", name="d")

                        def od_pair(jp, p2):
                            # O and D accumulation for pair jp (fp8 DoubleRow)
                            vt_dr = vt[h][:, jp * 256:(jp + 1) * 256].rearrange(
                                "p (j d) -> p j d", j=2)
                            for rr in range(NR):
                                psl = p2[:, :, rr * 512:(rr + 1) * 512]
                                nc.tensor.matmul(
                                    o_ps[:, rr * 512:(rr + 1) * 512],
                                    vt_dr, psl,
                                    start=(jp == 0), stop=(jp == NJP - 1),
                                    perf_mode=DR)
                                nc.tensor.matmul(
                                    d_ps[:, rr * 512:(rr + 1) * 512],
                                    ones_dr8[:], psl,
                                    start=(jp == 0), stop=(jp == NJP - 1),
                                    perf_mode=DR)

                        # software pipeline: emit O/D of pair jp-1 after the S
                        # matmuls of pair jp, so the in-order PE queue never
                        # heads into an O matmul whose exp isn't done yet
                        pending = None
                        for jp in range(NJP):
                            p2 = pexp.tile([128, 2, IW], f8, tag="p", name="p")
                            for jbi in range(2):
                                jb = jp * 2 + jbi
                                s_ps = psS.tile([128, IW], f32, tag="s", name="s")
                                for rr in range(NR):
                                    nc.tensor.matmul(
                                        s_ps[:, rr * 512:(rr + 1) * 512],
                                        kt[h][:, jb * 128:(jb + 1) * 128],
                                        qt[h][:, i0 + rr * 512:i0 + (rr + 1) * 512],
                                        start=True, stop=True)
                                nc.scalar.activation(p2[:, jbi, :], s_ps[:],
                                                     AF.Exp, scale=ATT_SCALE)
                            if pending is not None:
                                od_pair(*pending)
                            pending = (jp, p2)
                        od_pair(*pending)
                        dinv = drow_pool.tile([1, IW], f32, tag="dinv", name="dinv")
                        nc.vector.reciprocal_approx_fast(out=dinv[:], in_=d_ps[0:1, :])
                        # bcast 1/D across partitions (Pool) and normalize O
                        bc = drow_pool.tile([128, IW], f32, tag="bcD", name="bcD")
                        nc.gpsimd.partition_broadcast(bc[:], dinv[:])
                        nc.vector.tensor_tensor(
                            osb[h][:, i0:i0 + IW], o_ps[:], bc[:], op=OP.mult)

            # kqv stays open through C (SBUF fits); no pool barrier, so
            # phase C front work overlaps the attention tail
            # ======= Phase C: ResnetBlock (per-512-column pipeline) =======
            with (
                tc.tile_pool(name="back", bufs=1) as back,
                tc.tile_pool(name="brow", bufs=4) as brow,
                tc.tile_pool(name="tmp", bufs=8) as tmp,
                tc.tile_pool(name="psB2", bufs=3, space="PSUM") as psB2,
                tc.tile_pool(name="psBrow2", bufs=1, space="PSUM") as psBrow2,
            ):
                NCC = NQ // 512
                cat = [osb[0], osb[1], x16[0], x16[1]]  # fp16, use [:, :NQ]

                def sl(t, cc):
                    return t[:, cc * 512:(cc + 1) * 512]

                def stats_all(tiles, nch, tag):
                    # one [1, NQ] PSUM row of channel sums, one ln + one exp,
                    # one Pool broadcast to a [128, NQ] fp16 scale tile
                    srow = psBrow2.tile([1, NQ], f32, tag="srow", name="srow")
                    sqs = {}
                    for i, t in enumerate(tiles):
                        for cc in range(NCC):
                            s = tmp.tile([128, 512], f16, tag="sq", name="sq",
                                         bufs=16)
                            nc.vector.tensor_tensor(s[:], sl(t, cc), sl(t, cc),
                                                    op=OP.mult)
                            sqs[(i, cc)] = s
                    for cc in range(NCC):
                        for i in range(len(tiles)):
                            nc.tensor.matmul(srow[:, cc * 512:(cc + 1) * 512],
                                             ones_col16[:], sqs[(i, cc)][:],
                                             start=(i == 0),
                                             stop=(i == len(tiles) - 1))
                    lt = brow.tile([1, NQ], f32, tag="lnt", name="lnt")
                    nc.scalar.activation(lt[:], srow[:], AF.Ln, bias=EPS,
                                         scale=1.0 / nch)
                    iv = brow.tile([1, NQ], f16, tag="iv", name="iv")
                    nc.scalar.activation(iv[:], lt[:], AF.Exp, scale=-0.5)
                    bc = back.tile([128, NQ], f16, tag=f"bc{tag}",
                                   name=f"bc{tag}")
                    nc.gpsimd.partition_broadcast(bc[:], iv[:])
                    return bc

                # x_short convs first (independent of stats; keeps PE busy)
                xs = [back.tile([128, NQ], f32, tag=f"xs{mo}", name=f"xs{mo}")
                      for mo in range(CT)]
                for cc in range(NCC):
                    for mo in range(CT):
                        ps = psB2.tile([128, 512], f32, tag="conv", name="conv")
                        for kc in range(C2T):
                            nc.tensor.matmul(
                                ps[:], wsT[kc][:, mo * 128:(mo + 1) * 128],
                                sl(cat[kc], cc),
                                start=(kc == 0), stop=(kc == C2T - 1))
                        nc.vector.tensor_scalar(
                            sl(xs[mo], cc), ps[:],
                            RATIO * ISQ2, bsc[mo], op0=OP.mult, op1=OP.add)

                # r1 stats per chunk
                bc1r = stats_all([x16[0], x16[1], osb[0], osb[1]], 2 * C, "r1")
                bc1 = [bc1r[:, cc * 512:(cc + 1) * 512] for cc in range(NCC)]

                # gr1 = gelu(alpha_r1 * cat * invr1); h1 conv per chunk
                gr1 = [back.tile([128, NQ], f16, tag=f"gr1{ct}", name=f"gr1{ct}")
                       for ct in range(C2T)]
                h1 = [back.tile([128, NQ], f16, tag=f"h1{mo}", name=f"h1{mo}")
                      for mo in range(CT)]
                for cc in range(NCC):
                    for ct in range(C2T):
                        cn = tmp.tile([128, 512], f16, tag="cn", name="cn", bufs=6)
                        nc.vector.tensor_tensor(cn[:], sl(cat[ct], cc), bc1[cc],
                                                op=OP.mult)
                        nc.scalar.activation(sl(gr1[ct], cc), cn[:], AF.Gelu,
                                             scale=ar1[ct])
                    for mo in range(CT):
                        ps = psB2.tile([128, 512], f32, tag="conv", name="conv")
                        for kc in range(C2T):
                            nc.tensor.matmul(
                                ps[:], w1T[kc][:, mo * 128:(mo + 1) * 128],
                                sl(gr1[kc], cc),
                                start=(kc == 0), stop=(kc == C2T - 1))
                        nc.vector.tensor_scalar(
                            sl(h1[mo], cc), ps[:],
                            RATIO, b1[mo], op0=OP.mult, op1=OP.add)

                # r2 stats + gr2 + y per chunk
                bc2r = stats_all(h1, C, "r2")
                bc2 = [bc2r[:, cc * 512:(cc + 1) * 512] for cc in range(NCC)]
                gr2 = [back.tile([128, NQ], f16, tag=f"gr2{ct}", name=f"gr2{ct}")
                       for ct in range(CT)]
                yt = [back.tile([128, NQ], f32, tag=f"yt{mo}", name=f"yt{mo}")
                      for mo in range(CT)]
                for cc in range(NCC):
                    for ct in range(CT):
                        hn = tmp.tile([128, 512], f16, tag="cn", name="hn", bufs=6)
                        nc.vector.tensor_tensor(hn[:], sl(h1[ct], cc), bc2[cc],
                                                op=OP.mult)
                        nc.scalar.activation(sl(gr2[ct], cc), hn[:], AF.Gelu,
                                             scale=ar2[ct])
                    for mo in range(CT):
                        ps = psB2.tile([128, 512], f32, tag="conv", name="conv")
                        for kc in range(CT):
                            nc.tensor.matmul(
                                ps[:], w2T[kc][:, mo * 128:(mo + 1) * 128],
                                sl(gr2[kc], cc),
                                start=(kc == 0), stop=(kc == CT - 1))
                        nc.vector.scalar_tensor_tensor(
                            sl(yt[mo], cc), ps[:], RATIO * ISQ2,
                            sl(xs[mo], cc), op0=OP.mult, op1=OP.add)
                    if cc % 2 == 1:
                        for mo in range(CT):
                            nc.sync.dma_start(
                                y_d[mo * 128:(mo + 1) * 128,
                                    (cc - 1) * 512:(cc + 1) * 512],
                                yt[mo][:, (cc - 1) * 512:(cc + 1) * 512])
            kqv_stack.close()


_PROGRAM = None


def get_program():
    global _PROGRAM
    if _PROGRAM is None:
        _PROGRAM = build_program()
    return _PROGRAM


def make_in_maps(inputs):
    x = np.asarray(inputs["x"], np.float32).reshape(B, C, N)
    col = lambda v, n: np.asarray(v, np.float32).reshape(n, 1)
    tr16 = lambda w: np.ascontiguousarray(
        np.asarray(w, np.float32).T).astype(np.float16)
    wmats = {"wqT": tr16(inputs["Wq"]), "wkT": tr16(inputs["Wk"]),
             "wsT": tr16(inputs["Ws"]), "w1T": tr16(inputs["W1"]),
             "w2T": tr16(inputs["W2"])}
    wpack = np.concatenate(
        [wmats[nm][i * 128:(i + 1) * 128, :]
         for nm, n in W_ORDER for i in range(n)], axis=1)
    bcols = {"bq": col(inputs["bq"], C), "bk": col(inputs["bk"], C),
             "b1": col(inputs["b1"], C),
             "bsc": ((col(inputs["bs"], C).astype(np.float64) +
                      col(inputs["b2"], C).astype(np.float64)) * ISQ2
                     ).astype(np.float32),
             "aq": col(inputs["alpha_q"], C), "ak": col(inputs["alpha_k"], C),
             "ar1": col(inputs["alpha_r1"], 2 * C),
             "ar2": col(inputs["alpha_r2"], C)}
    bpack = np.concatenate(
        [bcols[nm][i * 128:(i + 1) * 128, :]
         for nm, n in B_ORDER for i in range(n)], axis=1)
    shared = {"wpack": np.ascontiguousarray(wpack),
              "bpack": np.ascontiguousarray(bpack.astype(np.float32))}
    in_maps = []
    for b in range(B):
        for half in range(2):
            xp = (np.ascontiguousarray(x[b]) if half == 0
                  else np.ascontiguousarray(np.roll(x[b], -NQ, axis=1)))
            in_maps.append({"x": xp, **shared})
    return in_maps


def assemble_output(results):
    y = np.empty((B, C, N), np.float32)
    for core, res in enumerate(results):
        b, half = core // 2, core % 2
        y[b][:, half * NQ:(half + 1) * NQ] = res["y"]
    return y.reshape(B, C, HW, HW)


def _patch_ldw_opt():
    from concourse import bass_utils
    if getattr(bass_utils, "_ldw_patched", False):
        return
    orig = bass_utils.run_command

    def patched(argv, **kw):
        argv = ["--enable-ldw-opt=true" if a == "--enable-ldw-opt=false" else a
                for a in argv]
        return orig(argv, **kw)

    bass_utils.run_command = patched
    bass_utils._ldw_patched = True


def kernel(**inputs):
    from concourse.bass_utils import run_bass_kernel_spmd

    if LDW_OPT:
        _patch_ldw_opt()
    nc = get_program()
    in_maps = make_in_maps(inputs)
    out = run_bass_kernel_spmd(nc, in_maps, core_ids=list(range(8)))
    return assemble_output(out.results)


if __name__ == "__main__":
    get_program()
    print("built ok")


# revision 25
# speedup vs baseline: 1.1675x; 1.1675x over previous
"""Trainium2 Bass kernel for nn_AttnAware (pixnorm->conv1x1 q/k attention + ResnetBlock).

Sharding: 8 cores = 4 batches x 2 query-halves. Each core receives its batch's
x [256, 4096] with pixel columns rotated so that its 2048 query pixels are the
first 2048 columns (attention is permutation-invariant over keys, and all
other ops are per-pixel). Single SPMD program, no collectives.

Per-core data layout: channels on partitions, pixels on free axis.
All PE matmuls run in 16/8-bit (1 cycle/row streaming and cheap LDWEIGHTS;
conv weights are converted to fp16 host-side and shipped in one packed DMA).
Attention works in the S^T orientation: S^T[j,i] tiles [128 keys, i-chunk]
computed as k_block^T @ q in fp16, exp on ACT (scale fused) writing fp8e4 P
pair-tiles, O^T accumulated with fp8 DoubleRow matmuls (two key blocks per
instruction, 0.5 cycles/row) against V^T pre-transposed per head on the PE.
The softmax denominator is accumulated on the PE as fp8 DoubleRow
ones-matmuls over the same P tiles, 1/D comes from the fast approximate DVE
reciprocal, is broadcast across partitions on the Pool engine, and O is
normalized inline per (head, i-pass). Pixelnorm row->tile broadcasts also go
through the Pool engine (partition_broadcast) instead of PE matmuls.
"""

import math
from contextlib import ExitStack

import numpy as np

import concourse.bass as bass
import concourse.mybir as mybir
import concourse.tile as tile
from concourse import bacc
from concourse.masks import make_identity

# ---------------- problem constants (hardcoded per contract) ----------------
B = 4
C = 256
HW = 64
N = HW * HW              # 4096 pixels
NQ = N // 2              # 2048 query pixels per core
NH = 2
HD = C // NH             # 128
CT = C // 128            # 2 channel tiles
C2T = 2 * C // 128       # 4 channel tiles for cat
JB = N // 128            # 32 key blocks
ATT_SCALE = HD ** -0.5
RATIO = 1.0 / (1.0 + 1e-8)   # PartialConv mask ratio (== 1.0f in fp32)
EPS = 1e-8
ISQ2 = 1.0 / math.sqrt(2.0)

# packed weight tile order (14 x [128, 256] fp16 column-tiles)
W_ORDER = [("wqT", CT), ("wkT", CT), ("wsT", C2T), ("w1T", C2T), ("w2T", CT)]
NWT = sum(n for _, n in W_ORDER)
# packed bias/alpha column order (18 x [128, 1] f32)
B_ORDER = [("bq", CT), ("bk", CT), ("b1", CT), ("bsc", CT), ("aq", CT),
           ("ak", CT), ("ar1", C2T), ("ar2", CT)]
NBC = sum(n for _, n in B_ORDER)

# ---------------- tuning knobs ----------------
IW = 1024                # i-columns per attention pass (PSUM S tile width)
LDW_OPT = False          # walrus ldw-opt is incompatible with 16/8-bit ldweights

f32 = mybir.dt.float32
f32r = mybir.dt.float32r
f16 = mybir.dt.float16
f8 = mybir.dt.float8e4
AF = mybir.ActivationFunctionType
OP = mybir.AluOpType
DR = mybir.MatmulPerfMode.DoubleRow


def build_program():
    nc = bacc.Bacc("TRN2", target_bir_lowering=False, debug=False)

    # register the pixnorm epsilon as a const AP usable as an ACT bias
    _eps_t = nc.alloc_sbuf_tensor(f"const-float32-{EPS}", [128, 1], f32)
    nc.gpsimd.memset(_eps_t.ap(), EPS)
    nc.const_aps.aps[(f32, EPS)] = _eps_t.ap()
    nc.all_engine_barrier()

    d = {}
    d["x"] = nc.dram_tensor("x", (C, N), f32, kind="ExternalInput").ap()
    d["wpack"] = nc.dram_tensor("wpack", (128, NWT * 256), f16,
                                kind="ExternalInput").ap()
    d["bpack"] = nc.dram_tensor("bpack", (128, NBC), f32,
                                kind="ExternalInput").ap()
    d["y"] = nc.dram_tensor("y", (C, NQ), f32, kind="ExternalOutput").ap()

    with tile.TileContext(nc) as tc:
        _body(tc, nc, d)
    nc.compile()
    return nc


def _body(tc, nc, d):
    x_d, y_d = d["x"], d["y"]

    with ExitStack() as top:
        const = top.enter_context(tc.tile_pool(name="const", bufs=1))
        wts = top.enter_context(tc.tile_pool(name="wts", bufs=1))

        ident16 = const.tile([128, 128], f16, tag="ident16", name="ident16")
        make_identity(nc, ident16[:])
        ones_col16 = const.tile([128, 1], f16, tag="ones_col16", name="ones_col16")
        nc.vector.memset(ones_col16[:], 1.0)
        ones_dr8 = const.tile([128, 2, 32], f8, tag="ones_dr8", name="ones_dr8")
        nc.vector.memset(ones_dr8[:], 1.0)

        # tensors that live into phase C
        with tc.tile_pool(name="oout", bufs=1) as oout:
            osb = [oout.tile([128, NQ], f16, tag=f"o{h}", name=f"o{h}")
                   for h in range(NH)]
            x16 = [oout.tile([128, N], f16, tag=f"x16_{h}", name=f"x16_{h}")
                   for h in range(NH)]

            # kqv: tensors that live from phase A through attention; closed
            # explicitly before the ResnetBlock pools open to reuse SBUF
            kqv_stack = ExitStack()
            kqv = kqv_stack.enter_context(tc.tile_pool(name="kqv", bufs=1))
            vt = [kqv.tile([128, N], f8, tag=f"vt{h}", name=f"vt{h}") for h in range(NH)]
            kt = [kqv.tile([128, N], f16, tag=f"k{h}", name=f"k{h}") for h in range(NH)]
            qt = [kqv.tile([128, NQ], f16, tag=f"q{h}", name=f"q{h}") for h in range(NH)]

            # x DMA first (critical path): 4 transfers, first pixel-half first
            front_stack = ExitStack()
            front = front_stack.enter_context(tc.tile_pool(name="front", bufs=1))
            xt = [front.tile([128, N], f32, tag=f"x{ct}", name=f"x{ct}")
                  for ct in range(CT)]
            # pixel half 0 on the SP ring, half 1 on the ACT ring (parallel)
            for ct in range(CT):
                nc.sync.dma_start(
                    xt[ct][:, :NQ], x_d[ct * 128:(ct + 1) * 128, :NQ])
            for ct in range(CT):
                nc.scalar.dma_start(
                    xt[ct][:, NQ:], x_d[ct * 128:(ct + 1) * 128, NQ:])

            # packed weights (one DMA) + packed biases/alphas (one DMA)
            wtile = wts.tile([128, NWT * 256], f16, tag="wpack", name="wpack")
            nc.sync.dma_start(wtile[:], d["wpack"])
            btile = wts.tile([128, NBC], f32, tag="bpack", name="bpack")
            nc.sync.dma_start(btile[:], d["bpack"])

            wv = {}
            off = 0
            for nm, n in W_ORDER:
                wv[nm] = [wtile[:, (off + i) * 256:(off + i + 1) * 256]
                          for i in range(n)]
                off += n
            bv = {}
            off = 0
            for nm, n in B_ORDER:
                bv[nm] = [btile[:, off + i:off + i + 1] for i in range(n)]
                off += n
            wqT, wkT, wsT, w1T, w2T = (wv[nm] for nm, _ in W_ORDER)
            bq, bk, b1, bsc, aq, ak, ar1, ar2 = (bv[nm] for nm, _ in B_ORDER)

            # =========== Phase A ===========
            with (
                tc.tile_pool(name="gtmp", bufs=4) as gtmp,
                tc.tile_pool(name="frow", bufs=2) as frow,
                tc.tile_pool(name="bcp", bufs=1) as bcp,
                tc.tile_pool(name="psA", bufs=2, space="PSUM") as psA,
                tc.tile_pool(name="psArow", bufs=1, space="PSUM") as psArow,
            ):
                # pixelnorm stats per half: squares (DVE, fp16), ones-matmul
                # column sums into one [1, 2048] PSUM row, one ln + one exp,
                # then one Pool broadcast to a [128, 2048] fp16 scale tile
                bch = []
                for half in range(2):
                    # fp16 copy of this half of x on the ACT engine (Identity,
                    # in every act table set), 1024-col chunks
                    for q in range(2):
                        for ct in range(CT):
                            qq = half * 2 + q
                            nc.scalar.activation(
                                x16[ct][:, qq * 1024:(qq + 1) * 1024],
                                xt[ct][:, qq * 1024:(qq + 1) * 1024],
                                AF.Identity, scale=1.0)
                    srow = psArow.tile([1, NQ], f32, tag="srow", name="srow")
                    for cc in range(4):
                        ccg = half * 4 + cc
                        for ct in range(CT):
                            s = gtmp.tile([128, 512], f16, tag="sq", name="sq",
                                          bufs=4)
                            nc.vector.tensor_tensor(
                                s[:], x16[ct][:, ccg * 512:(ccg + 1) * 512],
                                x16[ct][:, ccg * 512:(ccg + 1) * 512],
                                op=OP.mult)
                            nc.tensor.matmul(srow[:, cc * 512:(cc + 1) * 512],
                                             ones_col16[:], s[:],
                                             start=(ct == 0), stop=(ct == CT - 1))
                    lt = frow.tile([1, NQ], f32, tag="lnt", name="lnt")
                    nc.scalar.activation(lt[:], srow[:], AF.Ln, bias=EPS,
                                         scale=1.0 / C)
                    iv = frow.tile([1, NQ], f16, tag="inv", name="inv")
                    nc.scalar.activation(iv[:], lt[:], AF.Exp, scale=-0.5)
                    bc = bcp.tile([128, NQ], f16, tag=f"bc{half}",
                                  name=f"bc{half}")
                    nc.gpsimd.partition_broadcast(bc[:], iv[:])
                    bch.append(bc)

                # per-1024-chunk: xb = x*inv, gelu per conv, conv matmuls;
                # conv output moves alternate DVE / Pool to balance load
                def conv_block(c2, convs):
                    half, bc = c2 // 2, bch[c2 // 2]
                    csl = slice((c2 % 2) * 1024, (c2 % 2 + 1) * 1024)
                    gsl = slice(c2 * 1024, (c2 + 1) * 1024)
                    xb = []
                    for ct in range(CT):
                        t = gtmp.tile([128, 1024], f16, tag="g", name="xb")
                        nc.vector.tensor_tensor(
                            t[:], x16[ct][:, gsl], bc[:, csl], op=OP.mult)
                        xb.append(t)
                    for (wT, alpha, bias, out_tiles, oc0) in convs:
                        gchunks = []
                        for ct in range(CT):
                            g = gtmp.tile([128, 1024], f16, tag="g", name="g")
                            nc.scalar.activation(g[:], xb[ct][:],
                                                 AF.Gelu, scale=alpha[ct])
                            gchunks.append(g)
                        for mo in range(CT):
                            for rr in range(2):
                                ps = psA.tile([128, 512], f32, tag="conv",
                                              name="conv")
                                for kc in range(CT):
                                    nc.tensor.matmul(
                                        ps[:],
                                        wT[kc][:, mo * 128:(mo + 1) * 128],
                                        gchunks[kc][:, rr * 512:(rr + 1) * 512],
                                        start=(kc == 0), stop=(kc == CT - 1))
                                osl = slice(oc0 + (c2 % 2) * 1024 + rr * 512,
                                            oc0 + (c2 % 2) * 1024 + rr * 512 + 512)
                                if mo == 0:
                                    nc.vector.tensor_scalar(
                                        out_tiles[mo][:, osl], ps[:],
                                        bias[mo], None, op0=OP.add)
                                else:
                                    nc.scalar.activation(
                                        out_tiles[mo][:, osl], ps[:],
                                        AF.Identity, bias=bias[mo], scale=1.0)

                for c2 in range(2):
                    conv_block(c2, [(wqT, aq, bq, qt, 0), (wkT, ak, bk, kt, 0)])
                for c2 in range(2, 4):
                    conv_block(c2, [(wkT, ak, bk, kt, NQ)])

                # V^T per head: PE transpose of fp16 x, 4 blocks per PSUM slot,
                # copied out with a cast to fp8 (layout [(jb) d] flat == the
                # [jb/2, 2, d] DoubleRow view). Emitted after the convs so the
                # conv matmuls aren't stuck behind transposes that wait on the
                # second half of the x DMA.
                for h in range(NH):
                    for qb in range(JB // 4):
                        tp = psA.tile([128, 512], f16, tag="tp", name="tp")
                        for rr in range(4):
                            jb = qb * 4 + rr
                            nc.tensor.transpose(
                                tp[:, rr * 128:(rr + 1) * 128],
                                x16[h][:, jb * 128:(jb + 1) * 128], ident16[:])
                        nc.vector.tensor_copy(vt[h][:, qb * 512:(qb + 1) * 512], tp[:])

            front_stack.close()  # xt no longer needed (phase C uses x16)

            # =========== Phase B: attention (fp8 DoubleRow O and D) ==========
            with (
                tc.tile_pool(name="psS", bufs=2, space="PSUM") as psS,
                tc.tile_pool(name="psO", bufs=1, space="PSUM") as psO,
                tc.tile_pool(name="psD", bufs=1, space="PSUM") as psD,
                tc.tile_pool(name="pexp", bufs=3) as pexp,
                tc.tile_pool(name="drow", bufs=2) as drow_pool,
            ):
                NR = IW // 512
                NJP = JB // 2
                for h in range(NH):
                    for ip in range(NQ // IW):
                        i0 = ip * IW
                        o_ps = psO.tile([128, IW], f32, tag="o", name="o")
                        d_ps = psD.tile([32, IW], f32, tag="d", name="d")

                        def od_pair(jp, p2):
                            # O and D accumulation for pair jp (fp8 DoubleRow)
                            vt_dr = vt[h][:, jp * 256:(jp + 1) * 256].rearrange(
                                "p (j d) -> p j d", j=2)
                            for rr in range(NR):
                                psl = p2[:, :, rr * 512:(rr + 1) * 512]
                                nc.tensor.matmul(
                                    o_ps[:, rr * 512:(rr + 1) * 512],
                                    vt_dr, psl,
                                    start=(jp == 0), stop=(jp == NJP - 1),
                                    perf_mode=DR)
                                nc.tensor.matmul(
                                    d_ps[:, rr * 512:(rr + 1) * 512],
                                    ones_dr8[:], psl,
                                    start=(jp == 0), stop=(jp == NJP - 1),
                                    perf_mode=DR)

                        # software pipeline: emit O/D of pair jp-1 after the S
                        # matmuls of pair jp, so the in-order PE queue never
                        # heads into an O matmul whose exp isn't done yet
                        pending = None
                        for jp in range(NJP):
                            p2 = pexp.tile([128, 2, IW], f8, tag="p", name="p")
                            for jbi in range(2):
                                jb = jp * 2 + jbi
                                s_ps = psS.tile([128, IW], f32, tag="s", name="s")
                                for rr in range(NR):
                                    nc.tensor.matmul(
                                        s_ps[:, rr * 512:(rr + 1) * 512],
                                        kt[h][:, jb * 128:(jb + 1) * 128],
                                        qt[h][:, i0 + rr * 512:i0 + (rr + 1) * 512],
                                        start=True, stop=True)
                                nc.scalar.activation(p2[:, jbi, :], s_ps[:],
                                                     AF.Exp, scale=ATT_SCALE)
                            if pending is not None:
                                od_pair(*pending)
                            pending = (jp, p2)
                        od_pair(*pending)
                        dinv = drow_pool.tile([1, IW], f32, tag="dinv", name="dinv")
                        nc.vector.reciprocal_approx_fast(out=dinv[:], in_=d_ps[0:1, :])
                        # bcast 1/D across partitions (Pool) and normalize O
                        bc = drow_pool.tile([128, IW], f32, tag="bcD", name="bcD")
                        nc.gpsimd.partition_broadcast(bc[:], dinv[:])
                        nc.vector.tensor_tensor(
                            osb[h][:, i0:i0 + IW], o_ps[:], bc[:], op=OP.mult)

            # kqv pool (k/q/vt) closes here; back pool reuses its space
            kqv_stack.close()

            # ======= Phase C: ResnetBlock (per-512-column pipeline) =======
            with (
                tc.tile_pool(name="back", bufs=1) as back,
                tc.tile_pool(name="brow", bufs=4) as brow,
                tc.tile_pool(name="tmp", bufs=8) as tmp,
                tc.tile_pool(name="psB2", bufs=3, space="PSUM") as psB2,
                tc.tile_pool(name="psBrow2", bufs=1, space="PSUM") as psBrow2,
            ):
                NCC = NQ // 512
                cat = [osb[0], osb[1], x16[0], x16[1]]  # fp16, use [:, :NQ]

                def sl(t, cc):
                    return t[:, cc * 512:(cc + 1) * 512]

                def stats_all(tiles, nch, tag):
                    # one [1, NQ] PSUM row of channel sums, one ln + one exp,
                    # one Pool broadcast to a [128, NQ] fp16 scale tile
                    srow = psBrow2.tile([1, NQ], f32, tag="srow", name="srow")
                    for cc in range(NCC):
                        for i, t in enumerate(tiles):
                            s = tmp.tile([128, 512], f16, tag="sq", name="sq",
                                         bufs=8)
                            nc.vector.tensor_tensor(s[:], sl(t, cc), sl(t, cc),
                                                    op=OP.mult)
                            nc.tensor.matmul(srow[:, cc * 512:(cc + 1) * 512],
                                             ones_col16[:], s[:],
                                             start=(i == 0),
                                             stop=(i == len(tiles) - 1))
                    lt = brow.tile([1, NQ], f32, tag="lnt", name="lnt")
                    nc.scalar.activation(lt[:], srow[:], AF.Ln, bias=EPS,
                                         scale=1.0 / nch)
                    iv = brow.tile([1, NQ], f16, tag="iv", name="iv")
                    nc.scalar.activation(iv[:], lt[:], AF.Exp, scale=-0.5)
                    bc = back.tile([128, NQ], f16, tag=f"bc{tag}",
                                   name=f"bc{tag}")
                    nc.gpsimd.partition_broadcast(bc[:], iv[:])
                    return bc

                # x_short convs first (independent of stats; keeps PE busy)
                xs = [back.tile([128, NQ], f32, tag=f"xs{mo}", name=f"xs{mo}")
                      for mo in range(CT)]
                for cc in range(NCC):
                    for mo in range(CT):
                        ps = psB2.tile([128, 512], f32, tag="conv", name="conv")
                        for kc in range(C2T):
                            nc.tensor.matmul(
                                ps[:], wsT[kc][:, mo * 128:(mo + 1) * 128],
                                sl(cat[kc], cc),
                                start=(kc == 0), stop=(kc == C2T - 1))
                        nc.vector.tensor_scalar(
                            sl(xs[mo], cc), ps[:],
                            RATIO * ISQ2, bsc[mo], op0=OP.mult, op1=OP.add)

                # r1 stats per chunk
                bc1r = stats_all([x16[0], x16[1], osb[0], osb[1]], 2 * C, "r1")
                bc1 = [bc1r[:, cc * 512:(cc + 1) * 512] for cc in range(NCC)]

                # gr1 = gelu(alpha_r1 * cat * invr1); h1 conv per chunk
                gr1 = [back.tile([128, NQ], f16, tag=f"gr1{ct}", name=f"gr1{ct}")
                       for ct in range(C2T)]
                h1 = [back.tile([128, NQ], f16, tag=f"h1{mo}", name=f"h1{mo}")
                      for mo in range(CT)]
                for cc in range(NCC):
                    for ct in range(C2T):
                        cn = tmp.tile([128, 512], f16, tag="cn", name="cn", bufs=6)
                        nc.vector.tensor_tensor(cn[:], sl(cat[ct], cc), bc1[cc],
                                                op=OP.mult)
                        nc.scalar.activation(sl(gr1[ct], cc), cn[:], AF.Gelu,
                                             scale=ar1[ct])
                    for mo in range(CT):
                        ps = psB2.tile([128, 512], f32, tag="conv", name="conv")
                        for kc in range(C2T):
                            nc.tensor.matmul(
                                ps[:], w1T[kc][:, mo * 128:(mo + 1) * 128],
                                sl(gr1[kc], cc),
                                start=(kc == 0), stop=(kc == C2T - 1))
                        nc.vector.tensor_scalar(
                            sl(h1[mo], cc), ps[:],
                            RATIO, b1[mo], op0=OP.mult, op1=OP.add)

                # r2 stats + gr2 + y per chunk
                bc2r = stats_all(h1, C, "r2")
                bc2 = [bc2r[:, cc * 512:(cc + 1) * 512] for cc in range(NCC)]
                gr2 = [back.tile([128, NQ], f16, tag=f"gr2{ct}", name=f"gr2{ct}")
                       for ct in range(CT)]
                yt = [back.tile([128, NQ], f32, tag=f"yt{mo}", name=f"yt{mo}")
                      for mo in range(CT)]
                for cc in range(NCC):
                    for ct in range(CT):
                        hn = tmp.tile([128, 512], f16, tag="cn", name="hn", bufs=6)
                        nc.vector.tensor_tensor(hn[:], sl(h1[ct], cc), bc2[cc],
                                                op=OP.mult)
                        nc.scalar.activation(sl(gr2[ct], cc), hn[:], AF.Gelu,
                                             scale=ar2[ct])
                    for mo in range(CT):
                        ps = psB2.tile([128, 512], f32, tag="conv", name="conv")
                        for kc in range(CT):
                            nc.tensor.matmul(
                                ps[:], w2T[kc][:, mo * 128:(mo + 1) * 128],
                                sl(gr2[kc], cc),
                                start=(kc == 0), stop=(kc == CT - 1))
                        nc.vector.scalar_tensor_tensor(
                            sl(yt[mo], cc), ps[:], RATIO * ISQ2,
                            sl(xs[mo], cc), op0=OP.mult, op1=OP.add)
                    if cc % 2 == 1:
                        for mo in range(CT):
                            nc.sync.dma_start(
                                y_d[mo * 128:(mo + 1) * 128,
                                    (cc - 1) * 512:(cc + 1) * 512],
                                yt[mo][:, (cc - 1) * 512:(cc + 1) * 512])


_PROGRAM = None


def get_program():
    global _PROGRAM
    if _PROGRAM is None:
        _PROGRAM = build_program()
    return _PROGRAM


def make_in_maps(inputs):
    x = np.asarray(inputs["x"], np.float32).reshape(B, C, N)
    col = lambda v, n: np.asarray(v, np.float32).reshape(n, 1)
    tr16 = lambda w: np.ascontiguousarray(
        np.asarray(w, np.float32).T).astype(np.float16)
    wmats = {"wqT": tr16(inputs["Wq"]), "wkT": tr16(inputs["Wk"]),
             "wsT": tr16(inputs["Ws"]), "w1T": tr16(inputs["W1"]),
             "w2T": tr16(inputs["W2"])}
    wpack = np.concatenate(
        [wmats[nm][i * 128:(i + 1) * 128, :]
         for nm, n in W_ORDER for i in range(n)], axis=1)
    bcols = {"bq": col(inputs["bq"], C), "bk": col(inputs["bk"], C),
             "b1": col(inputs["b1"], C),
             "bsc": ((col(inputs["bs"], C).astype(np.float64) +
                      col(inputs["b2"], C).astype(np.float64)) * ISQ2
                     ).astype(np.float32),
             "aq": col(inputs["alpha_q"], C), "ak": col(inputs["alpha_k"], C),
             "ar1": col(inputs["alpha_r1"], 2 * C),
             "ar2": col(inputs["alpha_r2"], C)}
    bpack = np.concatenate(
        [bcols[nm][i * 128:(i + 1) * 128, :]
         for nm, n in B_ORDER for i in range(n)], axis=1)
    shared = {"wpack": np.ascontiguousarray(wpack),
              "bpack": np.ascontiguousarray(bpack.astype(np.float32))}
    in_maps = []
    for b in range(B):
        for half in range(2):
            xp = (np.ascontiguousarray(x[b]) if half == 0
                  else np.ascontiguousarray(np.roll(x[b], -NQ, axis=1)))
            in_maps.append({"x": xp, **shared})
    return in_maps


def assemble_output(results):
    y = np.empty((B, C, N), np.float32)
    for core, res in enumerate(results):
        b, half = core // 2, core % 2
        y[b][:, half * NQ:(half + 1) * NQ] = res["y"]
    return y.reshape(B, C, HW, HW)


def _patch_ldw_opt():
    from concourse import bass_utils
    if getattr(bass_utils, "_ldw_patched", False):
        return
    orig = bass_utils.run_command

    def patched(argv, **kw):
        argv = ["--enable-ldw-opt=true" if a == "--enable-ldw-opt=false" else a
                for a in argv]
        return orig(argv, **kw)

    bass_utils.run_command = patched
    bass_utils._ldw_patched = True


def kernel(**inputs):
    from concourse.bass_utils import run_bass_kernel_spmd

    if LDW_OPT:
        _patch_ldw_opt()
    nc = get_program()
    in_maps = make_in_maps(inputs)
    out = run_bass_kernel_spmd(nc, in_maps, core_ids=list(range(8)))
    return assemble_output(out.results)


if __name__ == "__main__":
    get_program()
    print("built ok")


# revision 26
# speedup vs baseline: 1.1856x; 1.0155x over previous
"""Trainium2 Bass kernel for nn_AttnAware (pixnorm->conv1x1 q/k attention + ResnetBlock).

Sharding: 8 cores = 4 batches x 2 query-halves. Each core receives its batch's
x [256, 4096] with pixel columns rotated so that its 2048 query pixels are the
first 2048 columns (attention is permutation-invariant over keys, and all
other ops are per-pixel). Single SPMD program, no collectives.

Per-core data layout: channels on partitions, pixels on free axis.
All PE matmuls run in 16/8-bit (1 cycle/row streaming and cheap LDWEIGHTS;
conv weights are converted to fp16 host-side and shipped in one packed DMA).
Attention works in the S^T orientation: S^T[j,i] tiles [128 keys, i-chunk]
computed as k_block^T @ q in fp16, exp on ACT (scale fused) writing fp8e4 P
pair-tiles, O^T accumulated with fp8 DoubleRow matmuls (two key blocks per
instruction, 0.5 cycles/row) against V^T pre-transposed per head on the PE.
The softmax denominator is accumulated on the PE as fp8 DoubleRow
ones-matmuls over the same P tiles, 1/D comes from the fast approximate DVE
reciprocal, is broadcast across partitions on the Pool engine, and O is
normalized inline per (head, i-pass). Pixelnorm row->tile broadcasts also go
through the Pool engine (partition_broadcast) instead of PE matmuls.
"""

import math
from contextlib import ExitStack

import numpy as np

import concourse.bass as bass
import concourse.mybir as mybir
import concourse.tile as tile
from concourse import bacc
from concourse.masks import make_identity

# ---------------- problem constants (hardcoded per contract) ----------------
B = 4
C = 256
HW = 64
N = HW * HW              # 4096 pixels
NQ = N // 2              # 2048 query pixels per core
NH = 2
HD = C // NH             # 128
CT = C // 128            # 2 channel tiles
C2T = 2 * C // 128       # 4 channel tiles for cat
JB = N // 128            # 32 key blocks
ATT_SCALE = HD ** -0.5
RATIO = 1.0 / (1.0 + 1e-8)   # PartialConv mask ratio (== 1.0f in fp32)
EPS = 1e-8
ISQ2 = 1.0 / math.sqrt(2.0)

# packed weight tile order (14 x [128, 256] fp16 column-tiles)
W_ORDER = [("wqT", CT), ("wkT", CT), ("wsT", C2T), ("w1T", C2T), ("w2T", CT)]
NWT = sum(n for _, n in W_ORDER)
# packed bias/alpha column order (18 x [128, 1] f32)
B_ORDER = [("bq", CT), ("bk", CT), ("b1", CT), ("bsc", CT), ("aq", CT),
           ("ak", CT), ("ar1", C2T), ("ar2", CT)]
NBC = sum(n for _, n in B_ORDER)

# ---------------- tuning knobs ----------------
IW = 1024                # i-columns per attention pass (PSUM S tile width)
LDW_OPT = False          # walrus ldw-opt is incompatible with 16/8-bit ldweights

f32 = mybir.dt.float32
f32r = mybir.dt.float32r
f16 = mybir.dt.float16
f8 = mybir.dt.float8e4
AF = mybir.ActivationFunctionType
OP = mybir.AluOpType
DR = mybir.MatmulPerfMode.DoubleRow


def build_program():
    nc = bacc.Bacc("TRN2", target_bir_lowering=False, debug=False)

    # register the pixnorm epsilon as a const AP usable as an ACT bias
    _eps_t = nc.alloc_sbuf_tensor(f"const-float32-{EPS}", [128, 1], f32)
    nc.gpsimd.memset(_eps_t.ap(), EPS)
    nc.const_aps.aps[(f32, EPS)] = _eps_t.ap()
    nc.all_engine_barrier()

    d = {}
    d["x"] = nc.dram_tensor("x", (C, N), f32, kind="ExternalInput").ap()
    d["wpack"] = nc.dram_tensor("wpack", (128, NWT * 256), f16,
                                kind="ExternalInput").ap()
    d["bpack"] = nc.dram_tensor("bpack", (128, NBC), f32,
                                kind="ExternalInput").ap()
    d["y"] = nc.dram_tensor("y", (C, NQ), f32, kind="ExternalOutput").ap()

    with tile.TileContext(nc) as tc:
        _body(tc, nc, d)
    nc.compile()
    return nc


def _body(tc, nc, d):
    x_d, y_d = d["x"], d["y"]

    with ExitStack() as top:
        const = top.enter_context(tc.tile_pool(name="const", bufs=1))
        wts = top.enter_context(tc.tile_pool(name="wts", bufs=1))

        ident16 = const.tile([128, 128], f16, tag="ident16", name="ident16")
        make_identity(nc, ident16[:])
        ones_col16 = const.tile([128, 1], f16, tag="ones_col16", name="ones_col16")
        nc.vector.memset(ones_col16[:], 1.0)
        ones_dr8 = const.tile([128, 2, 32], f8, tag="ones_dr8", name="ones_dr8")
        nc.vector.memset(ones_dr8[:], 1.0)

        # tensors that live into phase C
        with tc.tile_pool(name="oout", bufs=1) as oout:
            osb = [oout.tile([128, NQ], f16, tag=f"o{h}", name=f"o{h}")
                   for h in range(NH)]
            x16 = [oout.tile([128, N], f16, tag=f"x16_{h}", name=f"x16_{h}")
                   for h in range(NH)]

            # kqv: tensors that live from phase A through attention; closed
            # explicitly before the ResnetBlock pools open to reuse SBUF
            kqv_stack = ExitStack()
            kqv = kqv_stack.enter_context(tc.tile_pool(name="kqv", bufs=1))
            vt = [kqv.tile([128, N], f8, tag=f"vt{h}", name=f"vt{h}") for h in range(NH)]
            kt = [kqv.tile([128, N], f16, tag=f"k{h}", name=f"k{h}") for h in range(NH)]
            qt = [kqv.tile([128, NQ], f16, tag=f"q{h}", name=f"q{h}") for h in range(NH)]

            # x DMA first (critical path): 4 transfers, first pixel-half first
            front_stack = ExitStack()
            front = front_stack.enter_context(tc.tile_pool(name="front", bufs=1))
            xt = [front.tile([128, N], f32, tag=f"x{ct}", name=f"x{ct}")
                  for ct in range(CT)]
            # pixel half 0 on the SP ring, half 1 on the ACT ring (parallel)
            for ct in range(CT):
                nc.sync.dma_start(
                    xt[ct][:, :NQ], x_d[ct * 128:(ct + 1) * 128, :NQ])
            for ct in range(CT):
                nc.scalar.dma_start(
                    xt[ct][:, NQ:], x_d[ct * 128:(ct + 1) * 128, NQ:])

            # packed weights (one DMA) + packed biases/alphas (one DMA)
            wtile = wts.tile([128, NWT * 256], f16, tag="wpack", name="wpack")
            nc.sync.dma_start(wtile[:], d["wpack"])
            btile = wts.tile([128, NBC], f32, tag="bpack", name="bpack")
            nc.sync.dma_start(btile[:], d["bpack"])

            wv = {}
            off = 0
            for nm, n in W_ORDER:
                wv[nm] = [wtile[:, (off + i) * 256:(off + i + 1) * 256]
                          for i in range(n)]
                off += n
            bv = {}
            off = 0
            for nm, n in B_ORDER:
                bv[nm] = [btile[:, off + i:off + i + 1] for i in range(n)]
                off += n
            wqT, wkT, wsT, w1T, w2T = (wv[nm] for nm, _ in W_ORDER)
            bq, bk, b1, bsc, aq, ak, ar1, ar2 = (bv[nm] for nm, _ in B_ORDER)

            # =========== Phase A ===========
            with (
                tc.tile_pool(name="gtmp", bufs=4) as gtmp,
                tc.tile_pool(name="frow", bufs=2) as frow,
                tc.tile_pool(name="bcp", bufs=1) as bcp,
                tc.tile_pool(name="psA", bufs=2, space="PSUM") as psA,
                tc.tile_pool(name="psArow", bufs=1, space="PSUM") as psArow,
            ):
                # pixelnorm stats per half: squares (DVE, fp16), ones-matmul
                # column sums into one [1, 2048] PSUM row, one ln + one exp,
                # then one Pool broadcast to a [128, 2048] fp16 scale tile
                bch = []
                for half in range(2):
                    # fp16 copy of this half of x on the ACT engine (Identity,
                    # in every act table set), 1024-col chunks
                    for q in range(2):
                        for ct in range(CT):
                            qq = half * 2 + q
                            nc.scalar.activation(
                                x16[ct][:, qq * 1024:(qq + 1) * 1024],
                                xt[ct][:, qq * 1024:(qq + 1) * 1024],
                                AF.Identity, scale=1.0)
                    srow = psArow.tile([1, NQ], f32, tag="srow", name="srow")
                    for cc in range(4):
                        ccg = half * 4 + cc
                        for ct in range(CT):
                            s = gtmp.tile([128, 512], f16, tag="sq", name="sq",
                                          bufs=4)
                            nc.vector.tensor_tensor(
                                s[:], x16[ct][:, ccg * 512:(ccg + 1) * 512],
                                x16[ct][:, ccg * 512:(ccg + 1) * 512],
                                op=OP.mult)
                            nc.tensor.matmul(srow[:, cc * 512:(cc + 1) * 512],
                                             ones_col16[:], s[:],
                                             start=(ct == 0), stop=(ct == CT - 1))
                    lt = frow.tile([1, NQ], f32, tag="lnt", name="lnt")
                    nc.scalar.activation(lt[:], srow[:], AF.Ln, bias=EPS,
                                         scale=1.0 / C)
                    iv = frow.tile([1, NQ], f16, tag="inv", name="inv")
                    nc.scalar.activation(iv[:], lt[:], AF.Exp, scale=-0.5)
                    bc = bcp.tile([128, NQ], f16, tag=f"bc{half}",
                                  name=f"bc{half}")
                    nc.gpsimd.partition_broadcast(bc[:], iv[:])
                    bch.append(bc)

                # per-1024-chunk: xb = x*inv, gelu per conv, conv matmuls;
                # conv output moves alternate DVE / Pool to balance load
                def conv_block(c2, convs):
                    half, bc = c2 // 2, bch[c2 // 2]
                    csl = slice((c2 % 2) * 1024, (c2 % 2 + 1) * 1024)
                    gsl = slice(c2 * 1024, (c2 + 1) * 1024)
                    xb = []
                    for ct in range(CT):
                        t = gtmp.tile([128, 1024], f16, tag="g", name="xb")
                        nc.vector.tensor_tensor(
                            t[:], x16[ct][:, gsl], bc[:, csl], op=OP.mult)
                        xb.append(t)
                    for (wT, alpha, bias, out_tiles, oc0) in convs:
                        gchunks = []
                        for ct in range(CT):
                            g = gtmp.tile([128, 1024], f16, tag="g", name="g")
                            nc.scalar.activation(g[:], xb[ct][:],
                                                 AF.Gelu, scale=alpha[ct])
                            gchunks.append(g)
                        for mo in range(CT):
                            for rr in range(2):
                                ps = psA.tile([128, 512], f32, tag="conv",
                                              name="conv")
                                for kc in range(CT):
                                    nc.tensor.matmul(
                                        ps[:],
                                        wT[kc][:, mo * 128:(mo + 1) * 128],
                                        gchunks[kc][:, rr * 512:(rr + 1) * 512],
                                        start=(kc == 0), stop=(kc == CT - 1))
                                osl = slice(oc0 + (c2 % 2) * 1024 + rr * 512,
                                            oc0 + (c2 % 2) * 1024 + rr * 512 + 512)
                                nc.vector.tensor_scalar(
                                    out_tiles[mo][:, osl], ps[:],
                                    bias[mo], None, op0=OP.add)

                for c2 in range(2):
                    conv_block(c2, [(wqT, aq, bq, qt, 0), (wkT, ak, bk, kt, 0)])
                for c2 in range(2, 4):
                    conv_block(c2, [(wkT, ak, bk, kt, NQ)])

                # V^T per head: PE transpose of fp16 x, 4 blocks per PSUM slot,
                # copied out with a cast to fp8 (layout [(jb) d] flat == the
                # [jb/2, 2, d] DoubleRow view). Emitted after the convs so the
                # conv matmuls aren't stuck behind transposes that wait on the
                # second half of the x DMA.
                for h in range(NH):
                    for qb in range(JB // 4):
                        tp = psA.tile([128, 512], f16, tag="tp", name="tp")
                        for rr in range(4):
                            jb = qb * 4 + rr
                            nc.tensor.transpose(
                                tp[:, rr * 128:(rr + 1) * 128],
                                x16[h][:, jb * 128:(jb + 1) * 128], ident16[:])
                        nc.vector.tensor_copy(vt[h][:, qb * 512:(qb + 1) * 512], tp[:])

            front_stack.close()  # xt no longer needed (phase C uses x16)

            # =========== Phase B: attention (fp8 DoubleRow O and D) ==========
            with (
                tc.tile_pool(name="psS", bufs=2, space="PSUM") as psS,
                tc.tile_pool(name="psO", bufs=1, space="PSUM") as psO,
                tc.tile_pool(name="psD", bufs=1, space="PSUM") as psD,
                tc.tile_pool(name="pexp", bufs=3) as pexp,
                tc.tile_pool(name="drow", bufs=2) as drow_pool,
            ):
                NR = IW // 512
                NJP = JB // 2
                for h in range(NH):
                    for ip in range(NQ // IW):
                        i0 = ip * IW
                        o_ps = psO.tile([128, IW], f32, tag="o", name="o")
                        d_ps = psD.tile([32, IW], f32, tag="d", name="d")

                        def od_pair(jp, p2):
                            # O and D accumulation for pair jp (fp8 DoubleRow)
                            vt_dr = vt[h][:, jp * 256:(jp + 1) * 256].rearrange(
                                "p (j d) -> p j d", j=2)
                            for rr in range(NR):
                                psl = p2[:, :, rr * 512:(rr + 1) * 512]
                                nc.tensor.matmul(
                                    o_ps[:, rr * 512:(rr + 1) * 512],
                                    vt_dr, psl,
                                    start=(jp == 0), stop=(jp == NJP - 1),
                                    perf_mode=DR)
                                nc.tensor.matmul(
                                    d_ps[:, rr * 512:(rr + 1) * 512],
                                    ones_dr8[:], psl,
                                    start=(jp == 0), stop=(jp == NJP - 1),
                                    perf_mode=DR)

                        # software pipeline: emit O/D of pair jp-1 after the S
                        # matmuls of pair jp, so the in-order PE queue never
                        # heads into an O matmul whose exp isn't done yet
                        pending = None
                        for jp in range(NJP):
                            p2 = pexp.tile([128, 2, IW], f8, tag="p", name="p")
                            for jbi in range(2):
                                jb = jp * 2 + jbi
                                s_ps = psS.tile([128, IW], f32, tag="s", name="s")
                                for rr in range(NR):
                                    nc.tensor.matmul(
                                        s_ps[:, rr * 512:(rr + 1) * 512],
                                        kt[h][:, jb * 128:(jb + 1) * 128],
                                        qt[h][:, i0 + rr * 512:i0 + (rr + 1) * 512],
                                        start=True, stop=True)
                                nc.scalar.activation(p2[:, jbi, :], s_ps[:],
                                                     AF.Exp, scale=ATT_SCALE)
                            if pending is not None:
                                od_pair(*pending)
                            pending = (jp, p2)
                        od_pair(*pending)
                        dinv = drow_pool.tile([1, IW], f32, tag="dinv", name="dinv")
                        nc.vector.reciprocal_approx_fast(out=dinv[:], in_=d_ps[0:1, :])
                        # bcast 1/D across partitions (Pool) and normalize O
                        bc = drow_pool.tile([128, IW], f32, tag="bcD", name="bcD")
                        nc.gpsimd.partition_broadcast(bc[:], dinv[:])
                        nc.vector.tensor_tensor(
                            osb[h][:, i0:i0 + IW], o_ps[:], bc[:], op=OP.mult)

            # kqv pool (k/q/vt) closes here; back pool reuses its space
            kqv_stack.close()

            # ======= Phase C: ResnetBlock (per-512-column pipeline) =======
            with (
                tc.tile_pool(name="back", bufs=1) as back,
                tc.tile_pool(name="brow", bufs=4) as brow,
                tc.tile_pool(name="tmp", bufs=8) as tmp,
                tc.tile_pool(name="psB2", bufs=3, space="PSUM") as psB2,
                tc.tile_pool(name="psBrow2", bufs=1, space="PSUM") as psBrow2,
            ):
                NCC = NQ // 512
                cat = [osb[0], osb[1], x16[0], x16[1]]  # fp16, use [:, :NQ]

                def sl(t, cc):
                    return t[:, cc * 512:(cc + 1) * 512]

                def stats_all(tiles, nch, tag):
                    # one [1, NQ] PSUM row of channel sums, one ln + one exp,
                    # one Pool broadcast to a [128, NQ] fp16 scale tile
                    srow = psBrow2.tile([1, NQ], f32, tag="srow", name="srow")
                    for cc in range(NCC):
                        for i, t in enumerate(tiles):
                            s = tmp.tile([128, 512], f16, tag="sq", name="sq",
                                         bufs=8)
                            nc.vector.tensor_tensor(s[:], sl(t, cc), sl(t, cc),
                                                    op=OP.mult)
                            nc.tensor.matmul(srow[:, cc * 512:(cc + 1) * 512],
                                             ones_col16[:], s[:],
                                             start=(i == 0),
                                             stop=(i == len(tiles) - 1))
                    lt = brow.tile([1, NQ], f32, tag="lnt", name="lnt")
                    nc.scalar.activation(lt[:], srow[:], AF.Ln, bias=EPS,
                                         scale=1.0 / nch)
                    iv = brow.tile([1, NQ], f16, tag="iv", name="iv")
                    nc.scalar.activation(iv[:], lt[:], AF.Exp, scale=-0.5)
                    bc = back.tile([128, NQ], f16, tag=f"bc{tag}",
                                   name=f"bc{tag}")
                    nc.gpsimd.partition_broadcast(bc[:], iv[:])
                    return bc

                # x_short convs first (independent of stats; keeps PE busy)
                xs = [back.tile([128, NQ], f32, tag=f"xs{mo}", name=f"xs{mo}")
                      for mo in range(CT)]
                for cc in range(NCC):
                    for mo in range(CT):
                        ps = psB2.tile([128, 512], f32, tag="conv", name="conv")
                        for kc in range(C2T):
                            nc.tensor.matmul(
                                ps[:], wsT[kc][:, mo * 128:(mo + 1) * 128],
                                sl(cat[kc], cc),
                                start=(kc == 0), stop=(kc == C2T - 1))
                        nc.vector.tensor_scalar(
                            sl(xs[mo], cc), ps[:],
                            RATIO * ISQ2, bsc[mo], op0=OP.mult, op1=OP.add)

                # r1 stats per chunk
                bc1r = stats_all([x16[0], x16[1], osb[0], osb[1]], 2 * C, "r1")
                bc1 = [bc1r[:, cc * 512:(cc + 1) * 512] for cc in range(NCC)]

                # gr1 = gelu(alpha_r1 * cat * invr1); h1 conv per chunk
                gr1 = [back.tile([128, NQ], f16, tag=f"gr1{ct}", name=f"gr1{ct}")
                       for ct in range(C2T)]
                h1 = [back.tile([128, NQ], f16, tag=f"h1{mo}", name=f"h1{mo}")
                      for mo in range(CT)]
                for c2 in range(NCC // 2):
                    w1sl = slice(c2 * 1024, (c2 + 1) * 1024)
                    for ct in range(C2T):
                        cn = tmp.tile([128, 1024], f16, tag="cn", name="cn", bufs=6)
                        nc.vector.tensor_tensor(cn[:], cat[ct][:, w1sl],
                                                bc1r[:, w1sl], op=OP.mult)
                        nc.scalar.activation(gr1[ct][:, w1sl], cn[:], AF.Gelu,
                                             scale=ar1[ct])
                    for cc in (2 * c2, 2 * c2 + 1):
                        for mo in range(CT):
                            ps = psB2.tile([128, 512], f32, tag="conv", name="conv")
                            for kc in range(C2T):
                                nc.tensor.matmul(
                                    ps[:], w1T[kc][:, mo * 128:(mo + 1) * 128],
                                    sl(gr1[kc], cc),
                                    start=(kc == 0), stop=(kc == C2T - 1))
                            nc.vector.tensor_scalar(
                                sl(h1[mo], cc), ps[:],
                                RATIO, b1[mo], op0=OP.mult, op1=OP.add)

                # r2 stats + gr2 + y per chunk
                bc2r = stats_all(h1, C, "r2")
                bc2 = [bc2r[:, cc * 512:(cc + 1) * 512] for cc in range(NCC)]
                gr2 = [back.tile([128, NQ], f16, tag=f"gr2{ct}", name=f"gr2{ct}")
                       for ct in range(CT)]
                yt = [back.tile([128, NQ], f32, tag=f"yt{mo}", name=f"yt{mo}")
                      for mo in range(CT)]
                for c2 in range(NCC // 2):
                    w2sl = slice(c2 * 1024, (c2 + 1) * 1024)
                    for ct in range(CT):
                        hn = tmp.tile([128, 1024], f16, tag="cn", name="hn", bufs=6)
                        nc.vector.tensor_tensor(hn[:], h1[ct][:, w2sl],
                                                bc2r[:, w2sl], op=OP.mult)
                        nc.scalar.activation(gr2[ct][:, w2sl], hn[:], AF.Gelu,
                                             scale=ar2[ct])
                for cc in range(NCC):
                    for mo in range(CT):
                        ps = psB2.tile([128, 512], f32, tag="conv", name="conv")
                        for kc in range(CT):
                            nc.tensor.matmul(
                                ps[:], w2T[kc][:, mo * 128:(mo + 1) * 128],
                                sl(gr2[kc], cc),
                                start=(kc == 0), stop=(kc == CT - 1))
                        nc.vector.scalar_tensor_tensor(
                            sl(yt[mo], cc), ps[:], RATIO * ISQ2,
                            sl(xs[mo], cc), op0=OP.mult, op1=OP.add)
                    if cc % 2 == 1:
                        for mo in range(CT):
                            nc.sync.dma_start(
                                y_d[mo * 128:(mo + 1) * 128,
                                    (cc - 1) * 512:(cc + 1) * 512],
                                yt[mo][:, (cc - 1) * 512:(cc + 1) * 512])


_PROGRAM = None


def get_program():
    global _PROGRAM
    if _PROGRAM is None:
        _PROGRAM = build_program()
    return _PROGRAM


def make_in_maps(inputs):
    x = np.asarray(inputs["x"], np.float32).reshape(B, C, N)
    col = lambda v, n: np.asarray(v, np.float32).reshape(n, 1)
    tr16 = lambda w: np.ascontiguousarray(
        np.asarray(w, np.float32).T).astype(np.float16)
    wmats = {"wqT": tr16(inputs["Wq"]), "wkT": tr16(inputs["Wk"]),
             "wsT": tr16(inputs["Ws"]), "w1T": tr16(inputs["W1"]),
             "w2T": tr16(inputs["W2"])}
    wpack = np.concatenate(
        [wmats[nm][i * 128:(i + 1) * 128, :]
         for nm, n in W_ORDER for i in range(n)], axis=1)
    bcols = {"bq": col(inputs["bq"], C), "bk": col(inputs["bk"], C),
             "b1": col(inputs["b1"], C),
             "bsc": ((col(inputs["bs"], C).astype(np.float64) +
                      col(inputs["b2"], C).astype(np.float64)) * ISQ2
                     ).astype(np.float32),
             "aq": col(inputs["alpha_q"], C), "ak": col(inputs["alpha_k"], C),
             "ar1": col(inputs["alpha_r1"], 2 * C),
             "ar2": col(inputs["alpha_r2"], C)}
    bpack = np.concatenate(
        [bcols[nm][i * 128:(i + 1) * 128, :]
         for nm, n in B_ORDER for i in range(n)], axis=1)
    shared = {"wpack": np.ascontiguousarray(wpack),
              "bpack": np.ascontiguousarray(bpack.astype(np.float32))}
    in_maps = []
    for b in range(B):
        for half in range(2):
            xp = (np.ascontiguousarray(x[b]) if half == 0
                  else np.ascontiguousarray(np.roll(x[b], -NQ, axis=1)))
            in_maps.append({"x": xp, **shared})
    return in_maps


def assemble_output(results):
    y = np.empty((B, C, N), np.float32)
    for core, res in enumerate(results):
        b, half = core // 2, core % 2
        y[b][:, half * NQ:(half + 1) * NQ] = res["y"]
    return y.reshape(B, C, HW, HW)


def _patch_ldw_opt():
    from concourse import bass_utils
    if getattr(bass_utils, "_ldw_patched", False):
        return
    orig = bass_utils.run_command

    def patched(argv, **kw):
        argv = ["--enable-ldw-opt=true" if a == "--enable-ldw-opt=false" else a
                for a in argv]
        return orig(argv, **kw)

    bass_utils.run_command = patched
    bass_utils._ldw_patched = True


def kernel(**inputs):
    from concourse.bass_utils import run_bass_kernel_spmd

    if LDW_OPT:
        _patch_ldw_opt()
    nc = get_program()
    in_maps = make_in_maps(inputs)
    out = run_bass_kernel_spmd(nc, in_maps, core_ids=list(range(8)))
    return assemble_output(out.results)


if __name__ == "__main__":
    get_program()
    print("built ok")
